# revision 19
# baseline (speedup 1.0000x reference)
"""Trainium2 Bass kernel for nn_ContrastiveLossWithAttention.

Contract: kernel(**inputs) takes the FULL unsharded inputs (as produced by
reference.setup_inputs) and returns the FULL output (a float32 scalar).

Sharding: pure data parallel — batch dim B=16 split as 2 batches per core
across 8 NeuronCores. Each core reduces its two 2048x2048 pred slabs to two
per-row/per-col vectors (T1row, T1col); the host applies the O(B*N) scalar
epilogue and the final scalar reduction across cores.

Algorithm (validated vs the reference to ~1e-7 in fp64/fp32 numpy):
  gt_perm is a permutation ground truth: one 1 per valid row (identity
  restricted to rows i < src_ns here; verified exactly host-side, with a
  numpy fallback if the structure doesn't hold). Under that structure the
  loss collapses to threshold sums over pred alone:
    row_gt[i] = clip(p[i,i]) for i < s          (diagonal)
    src_pos   = row_gt^2
    T1row[i]  = sum_j 1{pred_c >= row_gt[i]-beta} * s2m[i,j]
    src_neg   = T1row - src_pos
    T1col[j]  = sum_i 1{pred_c >= col_gt[j]-beta} * s2m[i,j]   (col_gt == row_gt vec)
    corr      = sum_{j<s} (T1col[j] - col_gt[j]^2)
    loss_b    = -0.5 * sum_{i<s} [ln(src_pos_i) - ln(1 + src_neg_i + corr)]
  with pred_c = clip(pred,0,1)*1{j<t}, s2m = (pred_c * 1{i<s})^2.

Since corr only reads T1col[j] for j < src_ns <= 1536, the device computes
T1col on the first COLS=1536 columns only (the col-side DVE ops and the PE
column sums run at 3/4 width). T1row needs all 2048 columns.

Host prep is O(B*N) vector math + one clip/cast pass (sharding/padding):
  p16   = bf16(clip(pred,0,1)) with the ragged column tail [tgt_ns:] zeroed
  thr_r = f32 row thresholds  clip(diag)*rowmask - beta   (STT scalar operand)
  thrc  = bf16 of the same vector (column thresholds, broadcast on device)
Device does all O(N^2) work per 128-row chunk: the row-threshold sum (DVE
scalar_tensor_tensor with per-partition accumulate), the col indicator and
product (DVE tensor_tensor, bf16 2x mode), Square on ACT, and PE ones^T@
column sums accumulated in PSUM across the 12 chunks.
"""

import numpy as np
import ml_dtypes

B, N, M = 16, 2048, 2048
NCORES = 8
BPC = B // NCORES      # batches per core
PT = 128               # partitions
CH = N // PT           # row chunks per batch
CHR = 12               # row chunks actually computed: src_ns < 1537 always
                       # (setup_inputs range), so rows >= 1536 are never valid;
                       # guarded in kernel() with a fallback if violated
NR = PT * CHR          # rows computed on device
COLS = 1536            # columns for the T1col side: only j < src_ns <= 1536
                       # ever reach the epilogue's corr sum
NQ = COLS // 512       # 512-wide column slices for PE column sums

_cache = {}

# perf-mode opt-in per custom DVE op: 0 = 1x only, 1 = enable the
# hand-authored 2X_1PORT uop program (bf16 packed pairs)
PERF_MAX = 1
PERF_BY_OP = {"ANT_GE_SQ_TT": 1, "ANT_GE_SQ_ROW": 0, "ANT_GE_SQ_ROWE": 1}
# Row reduction path: the accumulator-bearing ROW op only works at 1x (no
# stock DVE op pairs accum with a 2x program either — likely a hw limit), so
# ROW_VIA_ACT uses an accum-free 2x row op + ACT Identity accumulate instead.
ROW_VIA_ACT = True


def _register_custom_dve_ops():
    """Register two fused DVE ops (runtime-registered; the uop table ships
    inside the NEFF so no firmware change is needed):
      ANT_GE_SQ_TT : out = (in0 >= in1) * in0^2           (col side)
      ANT_GE_SQ_ROW: out = (in0 >= s0) * in1 * in0,
                     accum_out = sum(out)  [call with in1 == in0]  (row side)
    Each gets a lower()-generated 1x program plus a hand-authored 2X_1PORT
    program (lo body in blocks 0-2, hi body in blocks 3-5; for the row op
    block 6 adds the pair and block 7 is the running accumulator, which is
    the pipe-tail convention DVE_READ_ACCUMULATOR expects)."""
    import concourse.dve_ops as dops

    if "ANT_GE_SQ_TT" in dops._SUB_OPCODE_FOR_NAME:
        return
    import numpy as np
    from operator import add as _add
    from concourse.dve_spec import Spec, Src0, Src1, C0, Zero, sq, lower
    from concourse.dve_uop import (
        DveOpSpec, UopConfig, UopDpConfig, InpSel, OutSel, OutPath,
        AluOp, AluInp, DelayInp, Trigger, ENABLE, DISABLE,
    )

    def _col_ref(in0, in1, s0, s1, imm2):
        x = in0.astype(np.float32)
        return (x >= in1.astype(np.float32)).astype(np.float32) * x * x

    def _row_ref(in0, in1, s0, s1, imm2):
        x = in0.astype(np.float32)
        t = np.asarray(s0, np.float32).reshape(-1, 1)
        body = ((x >= t).astype(np.float32) * in1.astype(np.float32) * x
                ).astype(np.float32)
        return body, body.reshape(body.shape[0], -1).sum(axis=-1, keepdims=True)

    col_spec = Spec(body=sq((Src0 >= Src1) * Src0), reference=_col_ref)
    row_spec = Spec(body=(Src0 >= C0) * Src1 * Src0, accum=_add,
                    accum_init=Zero, reference=_row_ref)

    def _passthrough(u, chains, blocks=range(8)):
        for k in blocks:
            u.datapath_config[k].pass_through_delay(*chains)

    def build_col_2x():
        u = UopConfig()
        _passthrough(u, (0, 1, 2, 3))
        b = u.datapath_config
        b[0].enable_alu(AluOp.IS_GE, AluInp.PREV_DELAY_0, AluInp.PREV_DELAY_1)
        b[1].enable_alu(AluOp.MULTIPLY, AluInp.PREV_ALU_OUT, AluInp.PREV_DELAY_0)
        b[2].enable_alu(AluOp.MULTIPLY, AluInp.PREV_ALU_OUT, AluInp.PREV_ALU_OUT)
        b[3].enable_alu(AluOp.IS_GE, AluInp.PREV_DELAY_2, AluInp.PREV_DELAY_3)
        b[3].enable_delay_from_src(DelayInp.PREV_ALU_OUT, 4)
        _passthrough(u, (4,), range(4, 8))
        b[4].enable_alu(AluOp.MULTIPLY, AluInp.PREV_ALU_OUT, AluInp.PREV_DELAY_2)
        b[5].enable_alu(AluOp.MULTIPLY, AluInp.PREV_ALU_OUT, AluInp.PREV_ALU_OUT)
        b[6].pass_through_alu()
        b[7].pass_through_alu()
        u.enable_input(InpSel.SRC_0, 1).enable_input(InpSel.SRC_1, 2)
        u.enable_input(InpSel.SRC_0_HI, 3).enable_input(InpSel.SRC_1_HI, 4)
        u.enable_output(OutSel.DELAY_4, OutPath.WR0_LO)
        u.enable_output(OutSel.ALU_OUT, OutPath.WR0_HI)
        u.require_inp0 = ENABLE
        u.require_inp1 = ENABLE
        u.trigger = (Trigger.SRC_TENSOR_DONE, Trigger.NONE, Trigger.NONE)
        return [u]

    def _row_2x_blocks(u):
        _passthrough(u, (0, 1, 2, 3))
        b = u.datapath_config
        b[0].enable_alu(AluOp.IS_GE, AluInp.PREV_DELAY_0, AluInp.PREV_DELAY_1)
        b[1].enable_alu(AluOp.MULTIPLY, AluInp.PREV_ALU_OUT, AluInp.PREV_DELAY_0)
        b[2].enable_alu(AluOp.MULTIPLY, AluInp.PREV_ALU_OUT, AluInp.PREV_ALU_OUT)
        b[3].enable_alu(AluOp.IS_GE, AluInp.PREV_DELAY_2, AluInp.PREV_DELAY_1)
        b[3].enable_delay_from_src(DelayInp.PREV_ALU_OUT, 4)
        _passthrough(u, (4,), range(4, 8))
        b[4].enable_alu(AluOp.MULTIPLY, AluInp.PREV_ALU_OUT, AluInp.PREV_DELAY_2)
        b[5].enable_alu(AluOp.MULTIPLY, AluInp.PREV_ALU_OUT, AluInp.PREV_ALU_OUT)
        b[6].enable_alu(AluOp.ADD, AluInp.PREV_ALU_OUT, AluInp.PREV_DELAY_4)
        b[6].enable_delay_from_src(DelayInp.PREV_ALU_OUT, 5)
        b[7].pass_through_delay(5)
        u.enable_input(InpSel.SRC_0, 1).enable_input(InpSel.CONST_0, 2)
        u.enable_input(InpSel.SRC_0_HI, 3).enable_input(InpSel.ZERO, 4)
        u.accum_enabled = ENABLE
        return b

    def build_row_2x():
        seed = UopConfig()
        b = _row_2x_blocks(seed)
        # acc <- 0 via x^x: input-independent, valid even while the delay
        # chains are still prefetching (the ZERO lane at depth 7 is not)
        b[7].enable_alu(AluOp.LOGICAL_XOR, AluInp.PREV_ALU_OUT,
                        AluInp.PREV_ALU_OUT)
        seed.require_inp0 = DISABLE
        seed.require_inp1 = DISABLE
        seed.repeat_count = 1
        seed.trigger = (Trigger.COUNT, Trigger.NONE, Trigger.NONE)
        seed.next_uop = (1, 0, 0)
        st = UopConfig()
        b = _row_2x_blocks(st)
        b[7].enable_alu(AluOp.ADD, AluInp.CURR_ALU_OUT, AluInp.PREV_ALU_OUT)
        st.enable_output(OutSel.DELAY_4, OutPath.WR0_LO)
        st.enable_output(OutSel.DELAY_5, OutPath.WR0_HI)
        st.require_inp0 = ENABLE
        st.require_inp1 = ENABLE
        st.trigger = (Trigger.SRC_TENSOR_DONE, Trigger.NONE, Trigger.NONE)
        return [seed, st]

    # Tile's re-emit pass reconstructs instructions, dropping a post-hoc
    # perf_max attribute — inject it at construction instead.
    if PERF_MAX:
        from concourse import bass_isa as _bisa
        _orig_ctor = _bisa.InstCustomDveAnt

        def _ctor_with_perf(*a, **kw):
            kw.setdefault("perf_max", PERF_BY_OP.get(kw.get("op_name"), 0))
            return _orig_ctor(*a, **kw)

        _bisa.InstCustomDveAnt = _ctor_with_perf

    # accum-free row op: same proven single-state 2x structure as the col op,
    # with the threshold on the CONST_0 lane instead of a second tensor
    rowe_spec = Spec(body=(Src0 >= C0) * Src1 * Src0,
                     reference=lambda in0, in1, s0, s1, imm2: (
                         (in0.astype(np.float32)
                          >= np.asarray(s0, np.float32).reshape(-1, 1))
                         * in1.astype(np.float32) * in0.astype(np.float32)))

    def build_rowe_2x():
        u = UopConfig()
        _passthrough(u, (0, 1, 2))
        b = u.datapath_config
        b[0].enable_alu(AluOp.IS_GE, AluInp.PREV_DELAY_0, AluInp.PREV_DELAY_1)
        b[1].enable_alu(AluOp.MULTIPLY, AluInp.PREV_ALU_OUT, AluInp.PREV_DELAY_0)
        b[2].enable_alu(AluOp.MULTIPLY, AluInp.PREV_ALU_OUT, AluInp.PREV_ALU_OUT)
        b[3].enable_alu(AluOp.IS_GE, AluInp.PREV_DELAY_2, AluInp.PREV_DELAY_1)
        b[3].enable_delay_from_src(DelayInp.PREV_ALU_OUT, 4)
        _passthrough(u, (4,), range(4, 8))
        b[4].enable_alu(AluOp.MULTIPLY, AluInp.PREV_ALU_OUT, AluInp.PREV_DELAY_2)
        b[5].enable_alu(AluOp.MULTIPLY, AluInp.PREV_ALU_OUT, AluInp.PREV_ALU_OUT)
        b[6].pass_through_alu()
        b[7].pass_through_alu()
        u.enable_input(InpSel.SRC_0, 1).enable_input(InpSel.CONST_0, 2)
        u.enable_input(InpSel.SRC_0_HI, 3)
        u.enable_output(OutSel.DELAY_4, OutPath.WR0_LO)
        u.enable_output(OutSel.ALU_OUT, OutPath.WR0_HI)
        u.require_inp0 = ENABLE
        u.require_inp1 = ENABLE
        u.trigger = (Trigger.SRC_TENSOR_DONE, Trigger.NONE, Trigger.NONE)
        return [u]

    entries = [
        ("ANT_GE_SQ_TT", col_spec, lower(col_spec, ver="v3"), build_col_2x()),
        ("ANT_GE_SQ_ROW", row_spec, lower(row_spec, ver="v3"), build_row_2x()),
        ("ANT_GE_SQ_ROWE", rowe_spec, lower(rowe_spec, ver="v3"), build_rowe_2x()),
    ]
    ops = {}
    for name, spec, u1x, u2x in entries:
        row = dops._CUSTOM_DVE_ROW_BASE + len(dops.OPS)
        op = dops.DveOp(name, spec, subdim=False, uops_sha={})
        dops.OPS.append(op)
        dops._SUB_OPCODE_FOR_NAME[name] = row
        dops.CUSTOM_DVE_SPECS[name] = spec
        spec_obj = DveOpSpec(name=name, opcode=row, uops=u1x, uops_2x=u2x,
                             rd1_en=True, perf_max=1)
        spec_obj.validate("v3")
        dops._COMPILE_CACHE[(name, "v3")] = spec_obj
        ops[name] = op
    return ops


def _get_custom_ops():
    import concourse.dve_ops as dops
    _register_custom_dve_ops()
    by_name = {op.name: op for op in dops.OPS}
    return (by_name["ANT_GE_SQ_TT"], by_name["ANT_GE_SQ_ROW"],
            by_name["ANT_GE_SQ_ROWE"])


def _build_program():
    import concourse.tile as tile
    from concourse import bacc, mybir

    f32 = mybir.dt.float32
    bf16 = mybir.dt.bfloat16
    Alu = mybir.AluOpType
    Act = mybir.ActivationFunctionType
    COL_OP, ROW_OP, ROWE_OP = _get_custom_ops()

    nc = bacc.Bacc("TRN2", debug=False, num_devices=NCORES)

    p_d = nc.dram_tensor("p16", [BPC, N, M], bf16, kind="ExternalInput")
    # thr_r is host-pretransposed to [PT, CHR] (p-major) for a contiguous DMA
    thr_d = nc.dram_tensor("thr_r", [BPC, NR], f32, kind="ExternalInput")
    thc_d = nc.dram_tensor("thrc16", [BPC, COLS], bf16, kind="ExternalInput")
    # t1row comes back [PT, CHR] p-major; host untransposes
    t1r_d = nc.dram_tensor("t1row", [BPC, NR], f32, kind="ExternalOutput")
    t1c_d = nc.dram_tensor("t1col", [BPC, COLS], f32, kind="ExternalOutput")

    with tile.TileContext(nc) as tc:
        with (
            tc.tile_pool(name="consts", bufs=1) as consts,
            tc.tile_pool(name="pb", bufs=2) as pb,
            tc.tile_pool(name="io", bufs=5) as io,
            tc.tile_pool(name="work", bufs=3) as work,
            tc.tile_pool(name="ps_bc", bufs=1, space="PSUM") as ps_bc,
            tc.tile_pool(name="ps_col", bufs=1, space="PSUM") as ps_col,
        ):
            ones16 = consts.tile([PT, 1], bf16, tag="ones16")
            nc.vector.memset(ones16, 1.0)
            ones_row = consts.tile([1, PT], bf16, tag="ones_row")
            nc.vector.memset(ones_row, 1.0)

            # per-batch thresholds: small DMAs on the scalar HWDGE queue so
            # they don't queue behind the big predc loads; thrc broadcast to
            # 128 partitions via a rank-1 PE matmul (ones^T @ thrc_row)
            thr_r = []
            thrc = []
            for b in range(BPC):
                t = consts.tile([PT, CHR], f32, tag=f"thr_r{b}")
                nc.scalar.dma_start(
                    out=t, in_=thr_d[b].rearrange("(p k) -> p k", p=PT)
                )
                thr_r.append(t)
                crow = consts.tile([1, COLS], bf16, tag=f"thrc_row{b}")
                nc.scalar.dma_start(out=crow, in_=thc_d[b:b + 1, :])
                bc_ps = ps_bc.tile([PT, COLS], f32, tag="bc")
                for q in range(NQ):
                    nc.tensor.matmul(
                        bc_ps[:, q * 512:(q + 1) * 512],
                        ones_row,
                        crow[0:1, q * 512:(q + 1) * 512],
                        start=True, stop=True,
                    )
                c = consts.tile([PT, COLS], bf16, tag=f"thrc{b}")
                nc.scalar.copy(c, bc_ps)
                thrc.append(c)

            for b in range(BPC):
                t1c_ps = ps_col.tile([1, COLS], f32, tag="t1col")
                t1row = pb.tile([PT, CHR], f32, tag="t1row")
                for k in range(CHR):
                    predc = io.tile([PT, M], bf16, tag="predc")
                    eng = nc.sync if k % 2 == 0 else nc.scalar
                    eng.dma_start(out=predc, in_=p_d[b, k * PT:(k + 1) * PT, :])
                    junk = work.tile([PT, M], bf16, tag="junk")
                    if ROW_VIA_ACT:
                        nc.vector._custom_dve(
                            ROWE_OP, out=junk, in0=predc, in1=predc,
                            s0=thr_r[b][:, k:k + 1],
                        )
                        junk2 = work.tile([PT, M], bf16, tag="junk2")
                        nc.scalar.activation(
                            out=junk2, in_=junk, func=Act.Identity,
                            accum_out=t1row[:, k:k + 1],
                        )
                    else:
                        nc.vector._custom_dve(
                            ROW_OP, out=junk, in0=predc, in1=predc,
                            s0=thr_r[b][:, k:k + 1], accum_out=t1row[:, k:k + 1],
                        )
                    tcol = work.tile([PT, COLS], bf16, tag="tcol")
                    inst = nc.vector._custom_dve(
                        COL_OP, out=tcol, in0=predc[:, :COLS], in1=thrc[b],
                    )
                    inst.perf_max = PERF_MAX
                    for q in range(NQ):
                        nc.tensor.matmul(
                            t1c_ps[0:1, q * 512:(q + 1) * 512],
                            ones16,
                            tcol[:, q * 512:(q + 1) * 512],
                            start=(k == 0), stop=(k == CHR - 1),
                        )

                t1c_row = pb.tile([1, COLS], f32, tag="t1c_row")
                nc.scalar.copy(t1c_row, t1c_ps[0:1, :])
                nc.scalar.dma_start(out=t1c_d[b:b + 1, :], in_=t1c_row)
                nc.sync.dma_start(
                    out=t1r_d[b].rearrange("(p k) -> p k", p=PT), in_=t1row
                )

    nc.compile()
    return nc


def _get_program():
    if "nc" not in _cache:
        _cache["nc"] = _build_program()
    return _cache["nc"]


def _gt_is_identity_perm(gt_perm, src_ns):
    """Exact check: gt_perm[b] == eye * (i < src_ns[b]), all entries in {0,1}."""
    if gt_perm.shape != (B, N, M):
        return False
    if gt_perm.min() < 0.0:
        return False
    i = np.arange(N)
    rowmask = (i[None, :] < src_ns[:, None]).astype(np.float32)  # [B, N]
    d = gt_perm[:, i, i]
    if not np.array_equal(d, rowmask):
        return False
    if not np.array_equal(gt_perm.sum(axis=2), rowmask):
        return False
    return True


def _reference_numpy(pred_dsmat, gt_perm, src_ns, tgt_ns, beta_value):
    """Direct numpy port of the reference — correctness fallback only."""
    out = 0.0
    n_sum = float(src_ns.astype(np.int64).sum())
    for b in range(pred_dsmat.shape[0]):
        p = pred_dsmat[b].astype(np.float64)
        g = gt_perm[b].astype(np.float64)
        s, t = int(src_ns[b]), int(tgt_ns[b])
        NN, MM = p.shape
        rm = (np.arange(NN) < s)
        cm = (np.arange(MM) < t)
        mask = rm[:, None] & cm[None, :]
        pred = np.clip(p, 0.0, 1.0) * mask
        gt = g * mask
        gp = pred * gt
        row_gt = gp.sum(1); col_gt = gp.sum(0)
        row_cnt = gt.sum(1); col_cnt = gt.sum(0)
        att_src = ((pred >= row_gt[:, None] - beta_value) & mask) * row_cnt[:, None]
        att_tgt = ((pred >= col_gt[None, :] - beta_value) & mask) * col_cnt[None, :]
        src_neg = (((att_src - gt) * pred) ** 2).sum(1)
        src_pos = (gp ** 2).sum(1)
        tgt_neg = (((att_tgt - gt) * pred) ** 2).sum(0)
        corr = (tgt_neg * col_cnt).sum()
        num = np.where(rm, src_pos, 1.0)
        den = np.where(rm, 1.0 + src_neg + corr, 1.0)
        out += -0.5 * (np.log(num / den) * rm).sum()
    return np.float32(out / n_sum)


def _host_prep(pred_dsmat, src_ns, tgt_ns, beta):
    ii = np.arange(N)
    rmask = (ii[None, :] < src_ns[:, None]).astype(np.float32)      # [B, N]
    diag = pred_dsmat[:, ii, ii].astype(np.float32)
    rowgt = np.clip(diag, 0.0, 1.0) * rmask                         # f32, exact
    srcpos = rowgt * rowgt
    thr = (rowgt - np.float32(beta)).astype(np.float32)             # [B, N]
    p16 = np.clip(pred_dsmat, 0.0, 1.0).astype(ml_dtypes.bfloat16)
    for gb in range(B):
        p16[gb, :, int(tgt_ns[gb]):] = 0                            # ragged col padding
        p16[gb, int(src_ns[gb]):, :] = 0                            # ragged row padding
    return rmask, srcpos, thr, p16


def _make_in_maps(p16, rmask, thr):
    thrc16 = thr.astype(ml_dtypes.bfloat16)
    # thr_r delivered p-major ([PT, CHR] flattened) for a contiguous DMA
    thr_pk = np.ascontiguousarray(
        thr[:, :NR].reshape(B, CHR, PT).transpose(0, 2, 1)
    ).reshape(B, NR)
    in_maps = []
    for c in range(NCORES):
        b0 = c * BPC
        in_maps.append({
            "p16": np.ascontiguousarray(p16[b0:b0 + BPC]),
            "thr_r": np.ascontiguousarray(thr_pk[b0:b0 + BPC]),
            "thrc16": np.ascontiguousarray(thrc16[b0:b0 + BPC, :COLS]),
        })
    return in_maps


def _host_epilogue(t1row, t1col, srcpos, rmask, src_ns):
    """O(B*N) scalar epilogue on the device-computed threshold sums."""
    t1row = t1row.astype(np.float64)
    t1col = t1col.astype(np.float64)
    srcpos = srcpos.astype(np.float64)
    rmask = rmask.astype(np.float64)
    corr = ((t1col - srcpos) * rmask).sum(axis=1)                   # [B]
    src_neg = t1row - srcpos
    num = np.where(rmask > 0, np.maximum(srcpos, 1e-300), 1.0)
    den = np.where(rmask > 0, 1.0 + src_neg + corr[:, None], 1.0)
    total = -0.5 * (np.log(num / den) * rmask).sum()
    n_sum = float(src_ns.astype(np.int64).sum())
    return np.float32(total / n_sum)


def kernel(pred_dsmat, gt_perm, src_ns, tgt_ns, beta_value):
    pred_dsmat = np.asarray(pred_dsmat, dtype=np.float32)
    gt_perm = np.asarray(gt_perm, dtype=np.float32)
    src_ns = np.asarray(src_ns, dtype=np.int32)
    tgt_ns = np.asarray(tgt_ns, dtype=np.int32)
    beta = float(np.asarray(beta_value))

    if not _gt_is_identity_perm(gt_perm, src_ns) or int(src_ns.max()) > NR:
        return _reference_numpy(pred_dsmat, gt_perm, src_ns, tgt_ns, beta)

    from concourse.bass_utils import run_bass_kernel_spmd

    nc = _get_program()
    rmask, srcpos, thr, p16 = _host_prep(pred_dsmat, src_ns, tgt_ns, beta)
    in_maps = _make_in_maps(p16, rmask, thr)
    res = run_bass_kernel_spmd(nc, in_maps, list(range(NCORES)))
    t1row_pk = np.concatenate([r["t1row"] for r in res.results], axis=0)  # [B, NR] p-major
    t1row_c = t1row_pk.reshape(B, PT, CHR).transpose(0, 2, 1).reshape(B, NR)
    t1row = np.zeros((B, N), np.float32)
    t1row[:, :NR] = t1row_c
    t1col_c = np.concatenate([r["t1col"] for r in res.results], axis=0)  # [B, COLS]
    t1col = np.zeros((B, M), np.float32)
    t1col[:, :COLS] = t1col_c
    return _host_epilogue(t1row, t1col, srcpos, rmask, src_ns)
